# revision 1
# baseline (speedup 1.0000x reference)
"""Trainium2 Bass kernel for nn_LinearSoftmaxAttention (second-order linear attention).

Math (per batch n, head h; L == S, D == M):
    Q = LN(queries)                       [L,D]
    K = LN(keys) / (3*sqrt(D)) * klen     [S,D]
    KV    = K^T V                         [D,M]
    Ksum  = sum_s K                       [D]
    KK    = K^T K                         [D,D]
    Qsum2 = Q^T Q                         [D,D]
    order1 = Q @ KV                       [L,M]
    norm1  = Q @ Ksum                     [L]
    norm2  = rowsum((Q @ KK) * Q)         [L]
    c      = rowsum((K @ Qsum2) * K)      [S]   (reference contracts QQ over l!)
    order2 = 0.5 * c[:,None] * V          [S,M]
    out = (order1 + order2) / (norm1 + 0.5*norm2)[:,None]

Sharding: one (n,h) pair per NeuronCore -> 8 heads over 8 cores, no collectives.
All per-core inputs are packed host-side into ONE contiguous [128, 388] array so
the kernel needs a single simple DMA (one logical DMA semaphore for consumers).
"""

from contextlib import ExitStack

import numpy as np

import concourse.bacc as bacc
import concourse.mybir as mybir
from concourse import tile
from concourse.bass_utils import run_bass_kernel_spmd
from concourse.masks import make_identity

# Problem constants (hardcoded per harness contract).
L = 512  # query length == key length
D = 32   # head dim == value dim
H = 8    # heads
P = 128  # SBUF partitions
T = L // P  # 4 row-chunks of 128
ALPHA = 3.0
LN_EPS = 1e-5
# K scale folded into the rsqrt: c = 1/(ALPHA*sqrt(D));  c/sqrt(v+eps) =
# 1/sqrt((v+eps)/c^2)  ->  Sqrt(scale=1/c^2, bias=eps/c^2) then reciprocal
_INV_C2 = ALPHA * ALPHA * D  # 1/c^2 = 288

# packed input layout (free-dim columns of the [P, NCOL] input): q|k|klen|v
_QOFF, _KOFF, _LOFF, _VOFF = 0, T * D, 2 * T * D, 2 * T * D + T
NCOL = 3 * T * D + T  # 388

_SUB = mybir.AluOpType.subtract
_MUL = mybir.AluOpType.mult
_ADD = mybir.AluOpType.add
_BYP = mybir.AluOpType.bypass


def _emit(ctx: ExitStack, tc: tile.TileContext, in_d, out_d):
    nc = tc.nc
    f32 = mybir.dt.float32
    X = mybir.AxisListType.X

    consts = ctx.enter_context(tc.tile_pool(name="consts", bufs=1))
    sbuf = ctx.enter_context(tc.tile_pool(name="sbuf", bufs=1))
    psum = ctx.enter_context(tc.tile_pool(name="psum", bufs=1, space="PSUM"))
    psum_tr = ctx.enter_context(tc.tile_pool(name="psum_tr", bufs=2, space="PSUM"))

    identity = consts.tile([P, P], f32)
    make_identity(nc, identity[:])
    eps_q = consts.tile([P, 1], f32)
    eps_k = consts.tile([P, 1], f32)
    dummy = consts.tile([P, 1], f32)
    # Dependency-free Sqrt issued first so Bacc's hoisted act-table load
    # overlaps the input DMA instead of sitting on the LN critical path.
    nc.scalar.activation(dummy[:], nc.const_aps.tensor(0.0, (P, 1)),
                         mybir.ActivationFunctionType.Sqrt)
    nc.gpsimd.memset(eps_q[:], LN_EPS)
    nc.gpsimd.memset(eps_k[:], _INV_C2 * LN_EPS)

    # ---- inputs [P, 388] = [q | k | klen | v]; two DMAs so LN (q,k,klen)
    # starts before v lands (v is only needed later, by phase A) ----
    in_all = sbuf.tile([P, NCOL], f32)
    nc.sync.dma_start(in_all[:, 0:_VOFF], in_d[:, 0:_VOFF])
    nc.scalar.dma_start(in_all[:, _VOFF:NCOL], in_d[:, _VOFF:NCOL])
    q_sb = in_all[:, _QOFF : _QOFF + T * D].rearrange("p (t d) -> p t d", d=D)
    k_sb = in_all[:, _KOFF : _KOFF + T * D].rearrange("p (t d) -> p t d", d=D)
    v_sb = in_all[:, _VOFF : _VOFF + T * D].rearrange("p (t d) -> p t d", d=D)
    klen_sb = in_all[:, _LOFF : _LOFF + T][:, :, None]  # [P, T, 1]

    # work holds [ r2 | Kn | r2 | Qn ] with r2 = sqrt(2) columns:
    # [r2|Kn] is the phase-A2 rhs (-> [r2*Ksum | KK]); [r2|Qn] feeds the nrm
    # reduction, whose 0.5 factor turns (r2*norm1)*r2 back into norm1 while
    # halving the u*Q term. This makes sb_A a single unscaled copy.
    R2 = float(np.sqrt(2.0))
    work = sbuf.tile([P, T, 2 * D + 2], f32)
    nc.gpsimd.memset(work[:, :, 0:1], R2)
    nc.gpsimd.memset(work[:, :, D + 1 : D + 2], R2)
    kn = work[:, :, 1 : D + 1]
    qn = work[:, :, D + 2 : 2 * D + 2]

    # ---- LayerNorm via bn_stats/bn_aggr + fused (x-mu)*rs apply ----
    def layernorm(src, dst_col, sqrt_scale, sqrt_bias, post=None, tag=""):
        stats = sbuf.tile([P, T, 6], f32, tag=f"ln_st{tag}")
        ag = sbuf.tile([P, T, 2], f32, tag=f"ln_ag{tag}")
        rs = sbuf.tile([P, T, 1], f32, tag=f"ln_rs{tag}")
        for t in range(T):
            nc.vector.bn_stats(stats[:, t, :], src[:, t, :])
            nc.vector.bn_aggr(ag[:, t, :], stats[:, t, :])
        # std = sqrt(var*scale + bias) on ACT; then rs = 1/std on DVE
        nc.scalar.activation(rs[:], ag[:, :, 1:2],
                             mybir.ActivationFunctionType.Sqrt,
                             scale=sqrt_scale, bias=sqrt_bias)
        nc.vector.reciprocal(rs[:], rs[:])
        if post is not None:
            nc.vector.tensor_mul(rs[:], rs[:], post)
        for t in range(T):
            nc.vector.tensor_scalar(
                out=work[:, t, dst_col : dst_col + D], in0=src[:, t, :],
                scalar1=ag[:, t, 0:1], scalar2=rs[:, t, 0:1],
                op0=_SUB, op1=_MUL)

    # k first: LN(k) -> phase A -> sb_A -> phase C is the longest chain
    layernorm(k_sb, 1, _INV_C2, eps_k[:], post=klen_sb, tag="k")
    layernorm(q_sb, D + 2, 1.0, eps_q[:], tag="q")

    # ---- qT [32, 512] via PE transpose + ACT copy (both idle here);
    # kT via DVE stream-transpose (32x32 blocks, SBUF->SBUF) ----
    qT = sbuf.tile([D, L], f32)
    kT = sbuf.tile([D, L], f32)
    for t in range(T):
        ptr = psum_tr.tile([D, P], f32, tag="ptr")
        nc.tensor.transpose(ptr[:], qn[:, t, :], identity[:])
        nc.scalar.copy(qT[:, t * P : (t + 1) * P], ptr[:])
    for t in range(T):
        for b in range(P // D):
            rows = slice(D * b, D * (b + 1))
            cols = slice(t * P + D * b, t * P + D * (b + 1))
            nc.vector.transpose(kT[:, cols], kn[rows, t, :])

    # ---- phase A/B: contract over s (accumulate 4 chunks in PSUM) ----
    # psumA[32, 0:D]   = sum_t Kn_t^T @ V_t         = KV
    # psumA[32, D:CAT] = sum_t Kn_t^T @ [r2 | Kn]_t = [r2*Ksum | KK]
    # psumB [32,32]    = sum_t Qn_t^T @ Qn_t        = Qsum2
    CAT = 2 * D + 1  # 65
    psumA1 = psum.tile([D, D], f32)
    psumA2 = psum.tile([D, D + 1], f32)
    psumB = psum.tile([D, D], f32)
    for t in range(T):
        st, sp = (t == 0), (t == T - 1)
        nc.tensor.matmul(psumA1[:], kn[:, t, :], v_sb[:, t, :],
                         start=st, stop=sp)
        nc.tensor.matmul(psumA2[:], kn[:, t, :], work[:, t, 0 : D + 1],
                         start=st, stop=sp)
        nc.tensor.matmul(psumB[:], qn[:, t, :], qn[:, t, :], start=st, stop=sp)

    # sb_A = [KV | r2*Ksum | KK] (unscaled copies), sb_B = 0.5*Qsum2
    sb_A = sbuf.tile([D, CAT], f32)
    sb_B = sbuf.tile([D, D], f32)
    nc.vector.tensor_copy(sb_A[:, 0:D], psumA1[:])
    nc.vector.tensor_copy(sb_A[:, D:CAT], psumA2[:])
    nc.vector.tensor_scalar(out=sb_B[:], in0=psumB[:], scalar1=0.5,
                            scalar2=None, op0=_MUL)

    # ---- phase C/D: contract over d ----
    # psumC[:, t, :] = Q_t @ [KV | Ksum | 0.5KK] = [order1 | norm1 | 0.5u]
    # psumD[:, t, :] = K_t @ (0.5*Qsum2)         = 0.5*t
    psumC = psum.tile([P, T, CAT], f32)
    psumD = psum.tile([P, T, D], f32)
    for t in range(T):
        nc.tensor.matmul(psumC[:, t, :], qT[:, t * P : (t + 1) * P], sb_A[:],
                         start=True, stop=True)
        nc.tensor.matmul(psumD[:, t, :], kT[:, t * P : (t + 1) * P],
                         sb_B[:], start=True, stop=True)

    # ---- epilogue: all ch/nrm reductions first (pipeline behind the
    # per-chunk C/D matmuls), one shared reciprocal, then final pairs ----
    out_sb = sbuf.tile([P, T, D], f32)
    ch = sbuf.tile([P, T, 1], f32)
    nrm = sbuf.tile([P, T, 1], f32)
    for t in range(T):
        s1 = sbuf.tile([P, D], f32, tag="epi_s1", bufs=2)
        s2 = sbuf.tile([P, D + 1], f32, tag="epi_s2", bufs=2)
        # ch_t = rowsum(0.5t * K)
        nc.vector.scalar_tensor_tensor(
            out=s1[:], in0=psumD[:, t, :], scalar=1.0,
            in1=kn[:, t, :], op0=_BYP, op1=_MUL, accum_out=ch[:, t, 0:1])
        # nrm_t = rowsum(0.5*[r2*norm1 | u] * [r2 | Q]) = norm1 + 0.5*norm2
        nc.vector.scalar_tensor_tensor(
            out=s2[:], in0=psumC[:, t, D:CAT], scalar=0.5,
            in1=work[:, t, D + 1 : 2 * D + 2], op0=_MUL, op1=_MUL,
            accum_out=nrm[:, t, 0:1])
    nc.vector.reciprocal(nrm[:], nrm[:])
    for t in range(T):
        s3 = sbuf.tile([P, D], f32, tag="epi_s3", bufs=2)
        # out_t = (V_t*ch_t + order1_t) * rnorm_t
        nc.vector.scalar_tensor_tensor(
            out=s3[:], in0=v_sb[:, t, :], scalar=ch[:, t, 0:1],
            in1=psumC[:, t, 0:D], op0=_MUL, op1=_ADD)
        nc.vector.tensor_scalar(out=out_sb[:, t, :], in0=s3[:],
                                scalar1=nrm[:, t, 0:1], scalar2=None, op0=_MUL)
    nc.sync.dma_start(out_d[:], out_sb[:].rearrange("p t d -> p (t d)"))


_CACHED = {}


def _build():
    if "nc" in _CACHED:
        return _CACHED["nc"]
    # Route every ACT func we use (Sqrt/Copy/Identity/Square) to the single
    # act-func-set that contains them all, so Bacc inserts ONE table load
    # instead of one per first-match set. Set ids are dict positions, which
    # this filter preserves.
    import concourse.hw_specs as hw_specs
    orig_tables = hw_specs.get_activation_tables

    def _tables_one_set(module_arch):
        tabs = orig_tables(module_arch)
        keep = None
        for name, funcs in tabs.items():
            names = {str(f) for f in funcs}
            if any("Sqrt" in s and "Rsqrt" not in s for s in names):
                keep = name
                break
        if keep is None:
            return tabs
        shared = {
            mybir.ActivationFunctionType.Copy,
            mybir.ActivationFunctionType.Identity,
            mybir.ActivationFunctionType.Square,
        }
        return {
            name: (funcs if name == keep else funcs - shared)
            for name, funcs in tabs.items()
        }

    bacc.get_activation_tables = _tables_one_set
    try:
        nc = bacc.Bacc("TRN2", target_bir_lowering=False, debug=False,
                       num_devices=H)
        f32 = mybir.dt.float32
        in_d = nc.dram_tensor("inp", [P, NCOL], f32, kind="ExternalInput")
        out_d = nc.dram_tensor("out", [P, T * D], f32, kind="ExternalOutput")
        with tile.TileContext(nc) as tc:
            with ExitStack() as ctx:
                _emit(ctx, tc, in_d[:], out_d[:])
        nc.compile()
    finally:
        bacc.get_activation_tables = orig_tables
    _CACHED["nc"] = nc
    return nc


def _pack(q, k, v, klen, h):
    # [512, 32] -> [128, 4*32] with col t*32+d = row t*128+p
    def rows(x):
        return np.ascontiguousarray(
            x.reshape(T, P, D).transpose(1, 0, 2).reshape(P, T * D))
    kl = np.ascontiguousarray(klen.reshape(T, P).T)  # [128, 4]
    return np.concatenate(
        [rows(q[0, :, h, :]), rows(k[0, :, h, :]), kl, rows(v[0, :, h, :])],
        axis=1).astype(np.float32)


def kernel(queries, keys, values, attn_mask, query_lengths, key_lengths,
           _want_profile=False, **_ignored):
    nc = _build()
    q = np.asarray(queries, dtype=np.float32)
    k = np.asarray(keys, dtype=np.float32)
    v = np.asarray(values, dtype=np.float32)
    klen = np.asarray(key_lengths, dtype=np.float32)

    in_maps = [{"inp": _pack(q, k, v, klen, h)} for h in range(H)]
    res = run_bass_kernel_spmd(nc, in_maps, list(range(H)),
                               trace=_want_profile)
    # [128, 128] -> [512, 32]
    outs = [
        res.results[h]["out"].reshape(P, T, D).transpose(1, 0, 2).reshape(L, D)
        for h in range(H)
    ]
    out = np.stack(outs, axis=1)[None]
    if _want_profile:
        return out.astype(np.float32), res
    return out.astype(np.float32)



# revision 2
# speedup vs baseline: 1.0537x; 1.0537x over previous
"""Trainium2 Bass kernel for nn_LinearSoftmaxAttention (second-order linear attention).

Math (per batch n, head h; L == S, D == M):
    Q = LN(queries)                       [L,D]
    K = LN(keys) / (3*sqrt(D)) * klen     [S,D]
    KV    = K^T V                         [D,M]
    Ksum  = sum_s K                       [D]
    KK    = K^T K                         [D,D]
    QQ    = Q^T Q                         [D,D]
    order1 = Q @ KV                       [L,M]
    norm1  = Q @ Ksum                     [L]
    u      = Q @ KK;  norm2 = rowsum(u * Q)
    tmat   = K @ (0.5*QQ); c = rowsum(tmat * K)
    order2 = c[:,None] * V
    out = (order1 + order2) / (norm1 + 0.5*norm2)[:,None]

Sharding: one (n,h) pair per NeuronCore -> 8 heads over 8 cores, no collectives.

v2 design notes (all matmul operands f16; fp32 would double every PE pass):
- LN stats via ONE grouped bn_stats per tensor + manual mean/var math
  (bn_stats 6-tuple per group: cnt/mean/M2 of even and odd elements).
- Apply uses broadcast (stride-0) APs: 2 tensor_tensor ops per tensor.
- Phase A/B is ONE matmul per row-chunk: stationary [kn|qn] [128,64],
  moving [1|v|kn|qn] [128,97] -> psumAB [64,97] holds every gram matrix
  (Ksum/KV/KK in kn rows, QQ in qn rows) accumulated over 4 chunks.
- PE transpose of [qn|kn] [128,64] -> [qnT;knT] [64,128] per chunk feeds a
  block-diagonal C/D matmul: lhsT=[qnT;knT], rhs=[sb_B | sb_A] [64,97]
  -> psumCD[:,t,:] = [0.5*tmat | norm1 | order1 | u] row-major.
- PE warm-up: 8 junk 512-col matmuls at kernel start keep the PE busy so the
  HAM clock-gate lifts (1.2 -> 2.4 GHz) before the real matmuls issue.
"""

from contextlib import ExitStack

import numpy as np

import concourse.bacc as bacc
import concourse.mybir as mybir
from concourse import tile
from concourse.bass_utils import run_bass_kernel_spmd
from concourse.masks import make_identity

# Problem constants (hardcoded per harness contract).
L = 512  # query length == key length
D = 32   # head dim == value dim
H = 8    # heads
P = 128  # SBUF partitions
T = L // P  # 4 row-chunks of 128
ALPHA = 3.0
LN_EPS = 1e-5
_INV_C2 = ALPHA * ALPHA * D  # 1/c^2 = 288 (K scale folded into sqrt)

_SUB = mybir.AluOpType.subtract
_MUL = mybir.AluOpType.mult
_ADD = mybir.AluOpType.add

# work tile free-dim layout: [1 | v | qn | kn | qn_dup | 2.0]
# [qn|kn] is the A/B stationary; [kn|qn_dup] is the transpose input;
# [kn|qn_dup|2.0] feeds ONE fused epilogue stt over [tmat|u|norm1]
_ONE, _V, _QN, _KN, _QN2, _TWO = 0, 1, 33, 65, 97, 129
WCOL = 130

KBYTES = 2 * T * D * 2 + T * 4  # k f16 + q f16 + klen f32 = 528


def _emit(ctx: ExitStack, tc: tile.TileContext, kin_d, v_d, out_d):
    nc = tc.nc
    f32 = mybir.dt.float32
    f16 = mybir.dt.float16
    u8 = mybir.dt.uint8
    X = mybir.AxisListType.X

    sbuf = ctx.enter_context(tc.tile_pool(name="sbuf", bufs=1))
    psum = ctx.enter_context(tc.tile_pool(name="psum", bufs=1, space="PSUM"))

    # ---- t=0: constants + PE warm-up (all independent of the input DMAs) ----
    wsrc = sbuf.tile([P, P], f16)
    nc.gpsimd.memset(wsrc[:], 0.5)
    # dummy act: hoists the ACT table load off the LN critical path
    dummy = sbuf.tile([P, 1], f32)
    nc.scalar.activation(dummy[:], nc.const_aps.tensor(0.0, (P, 1)),
                         mybir.ActivationFunctionType.Sqrt)
    psum_w = psum.tile([8, 512], f32)
    wrhs = wsrc[:, None, :].broadcast_to([P, 4, P])  # 512 junk cols
    for i in range(8):
        nc.tensor.matmul(psum_w[:], wsrc[:, 0:8], wrhs, start=True, stop=True)

    identity = sbuf.tile([P, P], f16)
    make_identity(nc, identity[:])
    eps_t = sbuf.tile([P, 1], f32)
    nc.gpsimd.memset(eps_t[:], LN_EPS)

    work = sbuf.tile([P, T, WCOL], f16)
    nc.gpsimd.memset(work[:, :, _ONE:_ONE + 1], 1.0)
    nc.gpsimd.memset(work[:, :, _TWO:_TWO + 1], 2.0)
    rhs_cd = sbuf.tile([64, 97], f16)
    nc.gpsimd.memset(rhs_cd[:], 0.0)

    # ---- input DMAs: k+q+klen in one transfer (sync), v (scalar) ----
    kin = sbuf.tile([P, KBYTES], u8)
    vraw = sbuf.tile([P, T, D], f16)
    nc.sync.dma_start(kin[:], kin_d[:], single_packet=True)
    nc.scalar.dma_start(vraw[:].rearrange("p t d -> p (t d)"), v_d[:], single_packet=True)
    # host packs [q | k | klen]; slot 0 = q, slot 1 = k
    kq = kin[:, 0:2 * T * D * 2].bitcast(f16).rearrange(
        "p (a t d) -> p a t d", a=2, d=D)
    klen = kin[:, 2 * T * D * 2:KBYTES].bitcast(f32)  # [P, T]

    # v -> work (gpsimd copy keeps ACT/DVE free)
    nc.gpsimd.tensor_copy(work[:, :, _V:_V + D], vraw[:])

    # ---- LayerNorm stats: grouped reduce + ACT square (k and q batched) ----
    # mean = sum/D;  var = sumsq/D - mean^2;  std' = sqrt(s*(var + eps))
    sq = sbuf.tile([P, 2, T, D], f32)
    nc.scalar.square(sq[:], kq)
    sums = sbuf.tile([P, 2, T], f32)
    nc.vector.reduce_sum(sums[:], kq, axis=X)
    ssq = sbuf.tile([P, 2, T], f32)
    nc.vector.reduce_sum(ssq[:], sq[:], axis=X)
    mu = sbuf.tile([P, 2, T], f32)
    nc.gpsimd.tensor_scalar(out=mu[:], in0=sums[:], scalar1=1.0 / D,
                            scalar2=None, op0=_MUL)
    m2 = sbuf.tile([P, 2, T], f32)  # sums^2 / D = D * mu^2
    nc.gpsimd.tensor_tensor(m2[:], sums[:], sums[:], _MUL)
    nc.gpsimd.tensor_scalar(out=m2[:], in0=m2[:], scalar1=1.0 / D,
                            scalar2=None, op0=_MUL)
    # centered q|k in one op (starts as soon as mu lands; hides under sqrt)
    qkc = sbuf.tile([P, T, 2, D], f16)
    nc.vector.tensor_tensor(
        qkc[:], kq.transpose([0, 2, 1, 3]),
        mu[:, :, :, None].transpose([0, 2, 1, 3]).broadcast_to([P, T, 2, D]),
        _SUB)
    var = sbuf.tile([P, 2, T], f32)  # D * actual variance
    nc.vector.tensor_tensor(var[:], ssq[:], m2[:], _SUB)
    std = sbuf.tile([P, 2, T], f32)
    nc.scalar.activation(std[:], var[:], mybir.ActivationFunctionType.Sqrt,
                         scale=1.0 / D, bias=eps_t[:])
    rs = sbuf.tile([P, 2, T], f32)
    nc.vector.reciprocal(rs[:], std[:])
    # rs_k *= klen / (alpha*sqrt(D))
    nc.gpsimd.tensor_scalar(out=rs[:, 0], in0=rs[:, 0],
                            scalar1=float(1.0 / np.sqrt(_INV_C2)),
                            scalar2=None, op0=_MUL)
    nc.gpsimd.tensor_tensor(rs[:, 0], rs[:, 0], klen, _MUL)

    # ---- apply: x_n = (x - mu)*rs; qn_dup copied on gpsimd (off-path) ----
    nc.vector.tensor_tensor(work[:, :, _QN:_QN + D], qc[:],
                            rs[:, 1, :, None].broadcast_to([P, T, D]), _MUL)
    nc.vector.tensor_tensor(work[:, :, _KN:_KN + D], kc[:],
                            rs[:, 0, :, None].broadcast_to([P, T, D]), _MUL)
    nc.gpsimd.tensor_copy(work[:, :, _QN2:_QN2 + D], work[:, :, _QN:_QN + D])

    # ---- phase A/B: one matmul per chunk; grams accumulate in psumAB ----
    # rows 0:32 = qn^T @ [1|v|qn|kn] = [. | QV | QQ | .]
    # rows 32:64 = kn^T @ ...        = [Ksum | KV | KQ | KK]
    psum_ab = psum.tile([64, 97], f32)
    for t in range(T):
        nc.tensor.matmul(psum_ab[:], work[:, t, _QN:_QN + 2 * D],
                         work[:, t, 0:97], start=(t == 0), stop=(t == T - 1))

    # ---- transposes: [kn|qn2] [128,64] -> [knT;qnT] [64,128] per chunk;
    # one psum tile, two DVE copies (2 chunks each) so C/D t0/t1 start early
    qkT = sbuf.tile([64, L], f16)
    ptr = psum.tile([64, T, P], f16)
    qkT4 = qkT[:].rearrange("a (t p) -> a t p", p=P)
    for t in range(T):
        nc.tensor.transpose(ptr[:, t, :], work[:, t, _KN:_KN + 2 * D],
                            identity[:])
    nc.vector.tensor_copy(qkT4[:], ptr[:])

    # ---- psumAB -> rhs_cd (f16): [0.5*QQ | KV | KK | Ksum] ----
    # D-block rows 0:32 (vs knT): cols 0:32 = 0.5*QQ
    # C-block rows 32:64 (vs qnT): cols 32:97 = c*[KV | c*KK | Ksum]
    nc.scalar.mul(rhs_cd[32:64, 0:32], psum_ab[32:64, 65:97], 0.5)
    nc.scalar.copy(rhs_cd[0:32, 32:96], psum_ab[0:32, 1:65])
    nc.scalar.copy(rhs_cd[0:32, 96:97], psum_ab[0:32, 0:1])

    # ---- phase C/D: one matmul per chunk ----
    # psumCD[:,t,:] = [order1(0:32) | tmat(32:64) | u(64:96) | norm1(96:97)]
    psum_cd = psum.tile([P, T, 97], f32)
    for t in range(T):
        nc.tensor.matmul(psum_cd[:, t, :], qkT[:, t * P:(t + 1) * P],
                         rhs_cd[:], start=True, stop=True)

    # ---- epilogue (row-major) ----
    # s = 0.5*[tmat|u|norm1] * [kn|qn2|2.0]  (one fused stt over 65 cols)
    # ch = rowsum(s[:,:32]);  nrm = rowsum(s[:,32:65]) = norm1 + 0.5*u.qn
    s = sbuf.tile([P, T, 2 * D + 1], f32)
    red = sbuf.tile([P, 2, T], f32)  # ch | nrm
    ch, nrm = red[:, 0], red[:, 1]
    nc.vector.scalar_tensor_tensor(out=s[:], in0=psum_cd[:, :, D:97],
                                   scalar=0.5, in1=work[:, :, _KN:_TWO + 1],
                                   op0=_MUL, op1=_MUL)
    nc.vector.reduce_sum(ch, s[:, :, 0:D], axis=X)
    nc.vector.reduce_sum(nrm, s[:, :, D:2 * D + 1], axis=X)
    nc.vector.reciprocal(nrm, nrm)
    # out = (order1 + ch*v) * rnorm
    m = sbuf.tile([P, T, D], f32)
    nc.gpsimd.tensor_tensor(m[:], vraw[:],
                            ch[:, :, None].broadcast_to([P, T, D]), _MUL)
    a = sbuf.tile([P, T, D], f32)
    nc.vector.tensor_tensor(a[:], m[:], psum_cd[:, :, 0:D], _ADD)
    out_sb = sbuf.tile([P, T, D], f32)
    nc.vector.tensor_tensor(out_sb[:], a[:],
                            nrm[:, :, None].broadcast_to([P, T, D]), _MUL)
    nc.sync.dma_start(out_d[:], out_sb[:].rearrange("p t d -> p (t d)"))


_CACHED = {}


def _build():
    if "nc" in _CACHED:
        return _CACHED["nc"]
    # Route every ACT func we use (Sqrt/Copy/Identity/Square) into the single
    # act-func-set containing Sqrt so Bacc inserts ONE table load.
    import concourse.hw_specs as hw_specs
    orig_tables = hw_specs.get_activation_tables

    def _tables_one_set(module_arch):
        tabs = orig_tables(module_arch)
        keep = None
        for name, funcs in tabs.items():
            names = {str(f) for f in funcs}
            if any("Sqrt" in s and "Rsqrt" not in s for s in names):
                keep = name
                break
        if keep is None:
            return tabs
        shared = {
            mybir.ActivationFunctionType.Copy,
            mybir.ActivationFunctionType.Identity,
            mybir.ActivationFunctionType.Square,
        }
        return {
            name: (funcs if name == keep else funcs - shared)
            for name, funcs in tabs.items()
        }

    bacc.get_activation_tables = _tables_one_set
    try:
        nc = bacc.Bacc("TRN2", target_bir_lowering=False, debug=False,
                       num_devices=H)
        f32 = mybir.dt.float32
        f16 = mybir.dt.float16
        u8 = mybir.dt.uint8
        kin_d = nc.dram_tensor("kin", [P, KBYTES], u8, kind="ExternalInput")
        v_d = nc.dram_tensor("vin", [P, T * D], f16, kind="ExternalInput")
        out_d = nc.dram_tensor("out", [P, T * D], f32, kind="ExternalOutput")
        with tile.TileContext(nc) as tc:
            with ExitStack() as ctx:
                _emit(ctx, tc, kin_d[:], v_d[:], out_d[:])
        nc.compile()
    finally:
        bacc.get_activation_tables = orig_tables
    _CACHED["nc"] = nc
    return nc


def _rows(x):
    # [512, 32] -> [128, 4*32] with col t*32+d = row t*128+p
    r = x.reshape(T, P, D).transpose(1, 0, 2)  # [P, T, D]
    return np.ascontiguousarray(r.reshape(P, T * D))


def _pack_maps(q, k, v, klen):
    maps = []
    for h in range(H):
        kb = _rows(k[0, :, h, :]).astype(np.float16)
        qb = _rows(q[0, :, h, :]).astype(np.float16)
        kl = np.ascontiguousarray(
            klen.reshape(T, P).T / (3.0 * np.sqrt(32.0))).astype(np.float32)
        kin = np.concatenate(
            [qb.view(np.uint8), kb.view(np.uint8), kl.view(np.uint8)], axis=1)
        maps.append({
            "kin": kin,
            "vin": _rows(v[0, :, h, :]).astype(np.float16),
        })
    return maps


def kernel(queries, keys, values, attn_mask, query_lengths, key_lengths,
           _want_profile=False, **_ignored):
    nc = _build()
    q = np.asarray(queries, dtype=np.float32)
    k = np.asarray(keys, dtype=np.float32)
    v = np.asarray(values, dtype=np.float32)
    klen = np.asarray(key_lengths, dtype=np.float32)

    in_maps = _pack_maps(q, k, v, klen)
    res = run_bass_kernel_spmd(nc, in_maps, list(range(H)),
                               trace=_want_profile)
    outs = [
        np.asarray(res.results[h]["out"]).astype(np.float32)
        .reshape(P, T, D).transpose(1, 0, 2).reshape(L, D)
        for h in range(H)
    ]
    out = np.stack(outs, axis=1)[None]
    if _want_profile:
        return out.astype(np.float32), res
    return out.astype(np.float32)


# revision 3
# speedup vs baseline: 1.0560x; 1.0022x over previous
"""Trainium2 Bass kernel for nn_LinearSoftmaxAttention (second-order linear attention).

Math (per batch n, head h; L == S, D == M):
    Q = LN(queries)                       [L,D]
    K = LN(keys) / (3*sqrt(D)) * klen     [S,D]
    KV    = K^T V                         [D,M]
    Ksum  = sum_s K                       [D]
    KK    = K^T K                         [D,D]
    QQ    = Q^T Q                         [D,D]
    order1 = Q @ KV                       [L,M]
    norm1  = Q @ Ksum                     [L]
    u      = Q @ KK;  norm2 = rowsum(u * Q)
    tmat   = K @ (0.5*QQ); c = rowsum(tmat * K)
    order2 = c[:,None] * V
    out = (order1 + order2) / (norm1 + 0.5*norm2)[:,None]

Sharding: one (n,h) pair per NeuronCore -> 8 heads over 8 cores, no collectives.

v2 design notes (all matmul operands f16; fp32 would double every PE pass):
- LN stats via ONE grouped bn_stats per tensor + manual mean/var math
  (bn_stats 6-tuple per group: cnt/mean/M2 of even and odd elements).
- Apply uses broadcast (stride-0) APs: 2 tensor_tensor ops per tensor.
- Phase A/B is ONE matmul per row-chunk: stationary [kn|qn] [128,64],
  moving [1|v|kn|qn] [128,97] -> psumAB [64,97] holds every gram matrix
  (Ksum/KV/KK in kn rows, QQ in qn rows) accumulated over 4 chunks.
- PE transpose of [qn|kn] [128,64] -> [qnT;knT] [64,128] per chunk feeds a
  block-diagonal C/D matmul: lhsT=[qnT;knT], rhs=[sb_B | sb_A] [64,97]
  -> psumCD[:,t,:] = [0.5*tmat | norm1 | order1 | u] row-major.
- PE warm-up: 8 junk 512-col matmuls at kernel start keep the PE busy so the
  HAM clock-gate lifts (1.2 -> 2.4 GHz) before the real matmuls issue.
"""

from contextlib import ExitStack

import numpy as np

import concourse.bacc as bacc
import concourse.mybir as mybir
from concourse import tile
from concourse.bass_utils import run_bass_kernel_spmd
from concourse.masks import make_identity

# Problem constants (hardcoded per harness contract).
L = 512  # query length == key length
D = 32   # head dim == value dim
H = 8    # heads
P = 128  # SBUF partitions
T = L // P  # 4 row-chunks of 128
ALPHA = 3.0
LN_EPS = 1e-5
_INV_C2 = ALPHA * ALPHA * D  # 1/c^2 = 288 (K scale folded into sqrt)

_SUB = mybir.AluOpType.subtract
_MUL = mybir.AluOpType.mult
_ADD = mybir.AluOpType.add

# work tile free-dim layout: [1 | v | qn | kn | qn_dup | 2.0]
# [qn|kn] is the A/B stationary; [kn|qn_dup] is the transpose input;
# [kn|qn_dup|2.0] feeds ONE fused epilogue stt over [tmat|u|norm1]
_ONE, _V, _QN, _KN, _QN2, _TWO = 0, 1, 33, 65, 97, 129
WCOL = 130

KBYTES = 2 * T * D * 2 + T * 4  # k f16 + q f16 + klen f32 = 528


def _emit(ctx: ExitStack, tc: tile.TileContext, kin_d, v_d, out_d):
    nc = tc.nc
    f32 = mybir.dt.float32
    f16 = mybir.dt.float16
    u8 = mybir.dt.uint8
    X = mybir.AxisListType.X

    sbuf = ctx.enter_context(tc.tile_pool(name="sbuf", bufs=1))
    psum = ctx.enter_context(tc.tile_pool(name="psum", bufs=1, space="PSUM"))

    # ---- t=0: constants + PE warm-up (all independent of the input DMAs) ----
    wsrc = sbuf.tile([P, P], f16)
    nc.gpsimd.memset(wsrc[:], 0.5)
    # dummy act: hoists the ACT table load off the LN critical path
    dummy = sbuf.tile([P, 1], f32)
    nc.scalar.activation(dummy[:], nc.const_aps.tensor(0.0, (P, 1)),
                         mybir.ActivationFunctionType.Sqrt)
    psum_w = psum.tile([8, 512], f32)
    wrhs = wsrc[:, None, :].broadcast_to([P, 4, P])  # 512 junk cols
    for i in range(8):
        nc.tensor.matmul(psum_w[:], wsrc[:, 0:8], wrhs, start=True, stop=True)

    identity = sbuf.tile([P, P], f16)
    make_identity(nc, identity[:])
    eps_t = sbuf.tile([P, 1], f32)
    nc.gpsimd.memset(eps_t[:], LN_EPS)

    work = sbuf.tile([P, T, WCOL], f16)
    nc.gpsimd.memset(work[:, :, _ONE:_ONE + 1], 1.0)
    nc.gpsimd.memset(work[:, :, _TWO:_TWO + 1], 2.0)
    rhs_cd = sbuf.tile([64, 97], f16)
    nc.gpsimd.memset(rhs_cd[:], 0.0)

    # ---- input DMAs: k+q+klen in one transfer (sync), v (scalar) ----
    kin = sbuf.tile([P, KBYTES], u8)
    vraw = sbuf.tile([P, T, D], f16)
    nc.sync.dma_start(kin[:], kin_d[:], single_packet=True)
    nc.scalar.dma_start(vraw[:].rearrange("p t d -> p (t d)"), v_d[:], single_packet=True)
    # host packs [q | k | klen]; slot 0 = q, slot 1 = k
    kq = kin[:, 0:2 * T * D * 2].bitcast(f16).rearrange(
        "p (a t d) -> p a t d", a=2, d=D)
    klen = kin[:, 2 * T * D * 2:KBYTES].bitcast(f32)  # [P, T]

    # v -> work (gpsimd copy keeps ACT/DVE free)
    nc.gpsimd.tensor_copy(work[:, :, _V:_V + D], vraw[:])

    # ---- LayerNorm stats: grouped reduce + ACT square (k and q batched) ----
    # mean = sum/D;  var = sumsq/D - mean^2;  std' = sqrt(s*(var + eps))
    sq = sbuf.tile([P, 2, T, D], f16)
    nc.scalar.square(sq[:], kq)
    sums = sbuf.tile([P, 2, T], f32)
    nc.vector.reduce_sum(sums[:], kq, axis=X)
    ssq = sbuf.tile([P, 2, T], f32)
    nc.vector.reduce_sum(ssq[:], sq[:], axis=X)
    mu = sbuf.tile([P, 2, T], f32)
    nc.gpsimd.tensor_scalar(out=mu[:], in0=sums[:], scalar1=1.0 / D,
                            scalar2=None, op0=_MUL)
    m2 = sbuf.tile([P, 2, T], f32)  # sums^2 / D = D * mu^2
    nc.gpsimd.tensor_tensor(m2[:], sums[:], sums[:], _MUL)
    nc.gpsimd.tensor_scalar(out=m2[:], in0=m2[:], scalar1=1.0 / D,
                            scalar2=None, op0=_MUL)
    # centered q|k in one op (starts as soon as mu lands; hides under sqrt)
    qkc = sbuf.tile([P, T, 2, D], f16)
    nc.vector.tensor_tensor(
        qkc[:], kq.transpose([0, 2, 1, 3]),
        mu[:, :, :, None].transpose([0, 2, 1, 3]).broadcast_to([P, T, 2, D]),
        _SUB)
    var = sbuf.tile([P, 2, T], f32)  # D * actual variance
    nc.vector.tensor_tensor(var[:], ssq[:], m2[:], _SUB)
    std = sbuf.tile([P, 2, T], f32)
    nc.scalar.activation(std[:], var[:], mybir.ActivationFunctionType.Sqrt,
                         scale=1.0 / D, bias=eps_t[:])
    rs = sbuf.tile([P, 2, T], f32)
    nc.vector.reciprocal(rs[:], std[:])
    # rs_k *= klen / (alpha*sqrt(D))
    nc.gpsimd.tensor_scalar(out=rs[:, 0], in0=rs[:, 0],
                            scalar1=float(1.0 / np.sqrt(_INV_C2)),
                            scalar2=None, op0=_MUL)
    nc.gpsimd.tensor_tensor(rs[:, 0], rs[:, 0], klen, _MUL)

    # ---- apply: x_n = (x - mu)*rs; qn_dup copied on gpsimd (off-path) ----
    nc.vector.tensor_tensor(work[:, :, _QN:_QN + D], qc[:],
                            rs[:, 1, :, None].broadcast_to([P, T, D]), _MUL)
    nc.vector.tensor_tensor(work[:, :, _KN:_KN + D], kc[:],
                            rs[:, 0, :, None].broadcast_to([P, T, D]), _MUL)
    nc.gpsimd.tensor_copy(work[:, :, _QN2:_QN2 + D], work[:, :, _QN:_QN + D])

    # ---- phase A/B: one matmul per chunk; grams accumulate in psumAB ----
    # rows 0:32 = qn^T @ [1|v|qn|kn] = [. | QV | QQ | .]
    # rows 32:64 = kn^T @ ...        = [Ksum | KV | KQ | KK]
    psum_ab = psum.tile([64, 97], f32)
    for t in range(T):
        nc.tensor.matmul(psum_ab[:], work[:, t, _QN:_QN + 2 * D],
                         work[:, t, 0:97], start=(t == 0), stop=(t == T - 1))

    # ---- transposes: [kn|qn2] [128,64] -> [knT;qnT] [64,128] per chunk;
    # one psum tile, two DVE copies (2 chunks each) so C/D t0/t1 start early
    qkT = sbuf.tile([64, L], f16)
    ptr = psum.tile([64, T, P], f16)
    qkT4 = qkT[:].rearrange("a (t p) -> a t p", p=P)
    for t in range(T):
        nc.tensor.transpose(ptr[:, t, :], work[:, t, _KN:_KN + 2 * D],
                            identity[:])
    nc.vector.tensor_copy(qkT4[:], ptr[:])

    # ---- psumAB -> rhs_cd (f16): [0.5*QQ | KV | KK | Ksum] ----
    # D-block rows 0:32 (vs knT): cols 0:32 = 0.5*QQ
    # C-block rows 32:64 (vs qnT): cols 32:97 = c*[KV | c*KK | Ksum]
    nc.scalar.mul(rhs_cd[32:64, 0:32], psum_ab[32:64, 65:97], 0.5)
    nc.scalar.copy(rhs_cd[0:32, 32:96], psum_ab[0:32, 1:65])
    nc.scalar.copy(rhs_cd[0:32, 96:97], psum_ab[0:32, 0:1])

    # ---- phase C/D: one matmul per chunk ----
    # psumCD[:,t,:] = [order1(0:32) | tmat(32:64) | u(64:96) | norm1(96:97)]
    psum_cd = psum.tile([P, T, 97], f32)
    for t in range(T):
        nc.tensor.matmul(psum_cd[:, t, :], qkT[:, t * P:(t + 1) * P],
                         rhs_cd[:], start=True, stop=True)

    # ---- epilogue (row-major) ----
    # s = 0.5*[tmat|u|norm1] * [kn|qn2|2.0]  (one fused stt over 65 cols)
    # ch = rowsum(s[:,:32]);  nrm = rowsum(s[:,32:65]) = norm1 + 0.5*u.qn
    s = sbuf.tile([P, T, 2 * D + 1], f32)
    red = sbuf.tile([P, 2, T], f32)  # ch | nrm
    ch, nrm = red[:, 0], red[:, 1]
    nc.vector.scalar_tensor_tensor(out=s[:], in0=psum_cd[:, :, D:97],
                                   scalar=0.5, in1=work[:, :, _KN:_TWO + 1],
                                   op0=_MUL, op1=_MUL)
    nc.vector.reduce_sum(ch, s[:, :, 0:D], axis=X)
    nc.vector.reduce_sum(nrm, s[:, :, D:2 * D + 1], axis=X)
    nc.vector.reciprocal(nrm, nrm)
    # out = (order1 + ch*v) * rnorm
    m = sbuf.tile([P, T, D], f32)
    nc.gpsimd.tensor_tensor(m[:], vraw[:],
                            ch[:, :, None].broadcast_to([P, T, D]), _MUL)
    a = sbuf.tile([P, T, D], f32)
    nc.vector.tensor_tensor(a[:], m[:], psum_cd[:, :, 0:D], _ADD)
    out_sb = sbuf.tile([P, T, D], f32)
    nc.vector.tensor_tensor(out_sb[:], a[:],
                            nrm[:, :, None].broadcast_to([P, T, D]), _MUL)
    nc.sync.dma_start(out_d[:], out_sb[:].rearrange("p t d -> p (t d)"))


_CACHED = {}


def _build():
    if "nc" in _CACHED:
        return _CACHED["nc"]
    # Route every ACT func we use (Sqrt/Copy/Identity/Square) into the single
    # act-func-set containing Sqrt so Bacc inserts ONE table load.
    import concourse.hw_specs as hw_specs
    orig_tables = hw_specs.get_activation_tables

    def _tables_one_set(module_arch):
        tabs = orig_tables(module_arch)
        keep = None
        for name, funcs in tabs.items():
            names = {str(f) for f in funcs}
            if any("Sqrt" in s and "Rsqrt" not in s for s in names):
                keep = name
                break
        if keep is None:
            return tabs
        shared = {
            mybir.ActivationFunctionType.Copy,
            mybir.ActivationFunctionType.Identity,
            mybir.ActivationFunctionType.Square,
        }
        return {
            name: (funcs if name == keep else funcs - shared)
            for name, funcs in tabs.items()
        }

    bacc.get_activation_tables = _tables_one_set
    try:
        nc = bacc.Bacc("TRN2", target_bir_lowering=False, debug=False,
                       num_devices=H)
        f32 = mybir.dt.float32
        f16 = mybir.dt.float16
        u8 = mybir.dt.uint8
        kin_d = nc.dram_tensor("kin", [P, KBYTES], u8, kind="ExternalInput")
        v_d = nc.dram_tensor("vin", [P, T * D], f16, kind="ExternalInput")
        out_d = nc.dram_tensor("out", [P, T * D], f32, kind="ExternalOutput")
        with tile.TileContext(nc) as tc:
            with ExitStack() as ctx:
                _emit(ctx, tc, kin_d[:], v_d[:], out_d[:])
        nc.compile()
    finally:
        bacc.get_activation_tables = orig_tables
    _CACHED["nc"] = nc
    return nc


def _rows(x):
    # [512, 32] -> [128, 4*32] with col t*32+d = row t*128+p
    r = x.reshape(T, P, D).transpose(1, 0, 2)  # [P, T, D]
    return np.ascontiguousarray(r.reshape(P, T * D))


def _pack_maps(q, k, v, klen):
    maps = []
    for h in range(H):
        kb = _rows(k[0, :, h, :]).astype(np.float16)
        qb = _rows(q[0, :, h, :]).astype(np.float16)
        kl = np.ascontiguousarray(
            klen.reshape(T, P).T / (3.0 * np.sqrt(32.0))).astype(np.float32)
        kin = np.concatenate(
            [qb.view(np.uint8), kb.view(np.uint8), kl.view(np.uint8)], axis=1)
        maps.append({
            "kin": kin,
            "vin": _rows(v[0, :, h, :]).astype(np.float16),
        })
    return maps


def kernel(queries, keys, values, attn_mask, query_lengths, key_lengths,
           _want_profile=False, **_ignored):
    nc = _build()
    q = np.asarray(queries, dtype=np.float32)
    k = np.asarray(keys, dtype=np.float32)
    v = np.asarray(values, dtype=np.float32)
    klen = np.asarray(key_lengths, dtype=np.float32)

    in_maps = _pack_maps(q, k, v, klen)
    res = run_bass_kernel_spmd(nc, in_maps, list(range(H)),
                               trace=_want_profile)
    outs = [
        np.asarray(res.results[h]["out"]).astype(np.float32)
        .reshape(P, T, D).transpose(1, 0, 2).reshape(L, D)
        for h in range(H)
    ]
    out = np.stack(outs, axis=1)[None]
    if _want_profile:
        return out.astype(np.float32), res
    return out.astype(np.float32)
